# revision 60
# baseline (speedup 1.0000x reference)
"""Trainium2 Bass kernel for nn_CADense (context-adaptive low-rank dense layer).

Computes, for the full batch:
    s_mod = s + context @ w          # [B, R]
    low   = (data @ u) * s_mod       # [B, R]
    out   = relu(low @ v.T + 2*bias) # [B, UNITS]

Sharding: data-parallel over batch across 8 NeuronCores; u/s/v/w/bias
replicated. Each core runs the same Bass program on its 1024-row shard.

The kernel is DMA-bound at fp32 (22.5 MiB/core vs the ~358 GB/s per-core
HBM limit), so the whole datapath runs in bf16: inputs are cast host-side,
matmuls are bf16 with fp32 PSUM accumulation, and the output is stored as
bf16 and upcast host-side. That halves HBM traffic to ~11.3 MiB/core,
which sits right at the PE streaming floor (~31 us) — the ridge point.

All compute stays in the transposed domain:
    pd[r, b]   = (u.T @ data.T)[r, b]                 (PE, fp32 PSUM)
    smod[r, b] = s[r] + (w.T @ ctx.T)[r, b]           (PE + scalar add)
    lowT[r, b] = pd * smod                            (DVE, writes bf16)
    outT[m, b] = relu((vT.T @ lowT)[m, b] + 2*bias[m])
The output stays transposed so the 2*bias term is a per-partition scalar:
it rides for free inside the PSUM-evacuation op (scalar.activation bias,
or DVE tensor_scalar max/add), eliminating the 32 K=1 bias matmuls the
natural-layout version needs (~7 us of pure PE streaming waste).

Schedule notes:
- Input DMAs ride the sync HWDGE ring in first-consumption order (the SP
  sequencer has no other duties, so ring-full stalls are free there);
  s/w/ctx go first so the smod stage doubles as real PE warm-up work,
  and the tiny s/bias transfers sit mid-stream where their ~2us
  completion latency hides under the pipelined bulk loads.
- Batch-tile 1's rank stage interleaves with batch-tile 0's output stage
  in PE emission order so the PE never waits on the load stream's tail.
- Output stores ride the gpsimd SWDGE ring so they round-robin against
  pending loads at the SDMA packet level; the last two groups store per-
  m-chunk on the two HWDGE rings so the final SWDGE drain isn't gated
  by a late store receipt (it was a 5.6us drain otherwise).
- PSUM evacuation of the 32 output groups alternates between the scalar
  and vector engines so neither gates PSUM bank recycling; batch-tile
  1's output groups rotate over all 8 PSUM banks (pd/smod banks have
  retired by then).
- A few bf16 garbage matmuls pre-warm the HAM clock gate while the first
  loads stream in; keepers sprinkled into the first rank stage cover the
  initial DMA-paced bubbles.
"""

import os
import sys
from contextlib import ExitStack

import numpy as np
import ml_dtypes


def _ensure_concourse():
    try:
        import concourse  # noqa: F401
    except ImportError:
        for p in ("/opt/trn_rl_repo", "/root/.axon_site/_ro/trn_rl_repo"):
            if os.path.isdir(p) and p not in sys.path:
                sys.path.insert(0, p)


_ensure_concourse()

import concourse.tile as tile  # noqa: E402
from concourse import bacc, mybir  # noqa: E402
from concourse.bass_utils import run_bass_kernel_spmd  # noqa: E402

NCORES = 8
B, N_IN, UNITS, RANK, CCTX = 8192, 2048, 2048, 256, 512
NB = B // NCORES  # batch rows per core
P = 128
BT = 512  # batch tile (free dim of matmuls)
NBT = NB // BT  # 2 batch tiles per core
KC = N_IN // P  # 16 contraction chunks for data @ u
CC = CCTX // P  # 4 contraction chunks for context @ w
RC = RANK // P  # 2 rank chunks
MC = UNITS // P  # 16 output unit chunks (partition dim of outT)
N_WARMUP_MM = 11

F32 = mybir.dt.float32
BF16 = mybir.dt.bfloat16
NP_BF16 = np.dtype(ml_dtypes.bfloat16)


def _emit(nc, tc, ctx):
    d_dataT = nc.dram_tensor("dataT", [N_IN, NB], BF16, kind="ExternalInput")
    d_ctxT = nc.dram_tensor("ctxT", [CCTX, NB], BF16, kind="ExternalInput")
    d_u = nc.dram_tensor("u", [N_IN, RANK], BF16, kind="ExternalInput")
    d_s = nc.dram_tensor("s", [RANK], F32, kind="ExternalInput")
    d_vT = nc.dram_tensor("vT", [RANK, UNITS], BF16, kind="ExternalInput")
    d_w = nc.dram_tensor("w", [CCTX, RANK], BF16, kind="ExternalInput")
    d_bias = nc.dram_tensor("bias", [UNITS], F32, kind="ExternalInput")
    d_outT = nc.dram_tensor("outT", [UNITS, NB], BF16, kind="ExternalOutput")

    ap_dataT = d_dataT.ap().rearrange("(c p) b -> p c b", p=P)
    ap_ctxT = d_ctxT.ap().rearrange("(cc p) b -> p cc b", p=P)
    ap_u = d_u.ap().rearrange("(q p) r -> p q r", p=P)
    ap_w = d_w.ap().rearrange("(cc p) r -> p cc r", p=P)
    ap_vT = d_vT.ap().rearrange("(rc p) m -> p rc m", p=P)
    ap_outT = d_outT.ap().rearrange("(mc p) b -> p mc b", p=P)

    singles = ctx.enter_context(tc.tile_pool(name="singles", bufs=1))
    du_psum = ctx.enter_context(tc.tile_pool(name="du_psum", bufs=2, space="PSUM"))
    s_psum = ctx.enter_context(tc.tile_pool(name="s_psum", bufs=2, space="PSUM"))
    o_psum = ctx.enter_context(tc.tile_pool(name="o_psum", bufs=4, space="PSUM"))

    # HAM warm-up fodder: garbage bf16 matmuls while the first loads stream.
    wu_a = singles.tile([P, P], BF16)
    nc.vector.memset(wu_a[:], 1.0)
    wu_b = singles.tile([P, BT], BF16)
    nc.vector.memset(wu_b[:], 1.0)

    # ---- input DMA queue (sync HWDGE ring), in first-use order ----------
    # Tiny transfers (s/bias/w) ride mid-stream where their ~2us completion
    # latency hides under the pipelined bulk loads; putting them first costs
    # ~5us of dead time at the head.
    u_t = {}  # u_t[g][:, j, :] covers kc = 4*g + j
    for g in range(4):
        u_t[g] = singles.tile([P, 4, RANK], BF16, name=f"u{g}")
    # dataT chunks: the first batch-tile-0 chunk is split in 2-kc halves so
    # the DMA-paced rank stage starts on a finer-grained completion; the
    # rest are 4-kc pieces (finer everywhere costs descriptor overhead).
    D0_CHUNKS = [(0, 2), (2, 2), (4, 4), (8, 4), (12, 4)]  # (kc_lo, width)
    d_t = {}
    for g, (kc_lo, w_kc) in enumerate(D0_CHUNKS):
        d_t[(0, g)] = singles.tile([P, w_kc, BT], BF16, name=f"d0g{g}")
    for g in range(4):
        d_t[(1, g)] = singles.tile([P, 4, BT], BF16, name=f"d1g{g}")

    def data_chunk(bt, kc):
        if bt == 0:
            for g, (kc_lo, w_kc) in enumerate(D0_CHUNKS):
                if kc_lo <= kc < kc_lo + w_kc:
                    return d_t[(0, g)][:, kc - kc_lo, :]
        return d_t[(1, kc // 4)][:, kc % 4, :]

    # All loads on the sync (SP) HWDGE ring: the SP sequencer has no other
    # duties, so it can absorb ring-full stalls that would block the ACT
    # engine's compute FIFO.
    def _ld_eng():
        return nc.sync

    def load_data(bt, g):
        if bt == 0:
            kc_lo, w_kc = D0_CHUNKS[g]
        else:
            kc_lo, w_kc = 4 * g, 4
        _ld_eng().dma_start(
            out=d_t[(bt, g)][:],
            in_=ap_dataT[:, kc_lo : kc_lo + w_kc, bt * BT : (bt + 1) * BT],
        )

    s_sb = singles.tile([P, RC], F32, name="s_sb")
    bias_sb = singles.tile([P, MC], F32, name="bias_sb")
    w_sb = singles.tile([P, CC, RANK], BF16, name="w_sb")
    # ctx(bt0) in two 2-cc halves so the smod stage can start on the first
    # half's arrival; ctx(bt1) rides mid-stream as one piece.
    ctx0_t = {
        h: singles.tile([P, 2, BT], BF16, name=f"ctx0h{h}") for h in range(2)
    }
    ctx1 = singles.tile([P, CC, BT], BF16, name="ctx1")
    vT_sb = singles.tile([P, RC, UNITS], BF16, name="vT_sb")

    def ctx_chunk(bt, cc):
        if bt == 0:
            return ctx0_t[cc // 2][:, cc % 2, :]
        return ctx1[:, cc, :]

    # w/ctx(bt0) first: the smod stage becomes real warm-up work for the
    # PE while the data stream ramps up; the tiny s load rides mid-stream
    # (needed only at the smod PSUM evacuation).
    _ld_eng().dma_start(out=w_sb[:], in_=ap_w)
    _ld_eng().dma_start(out=ctx0_t[0][:], in_=ap_ctxT[:, 0:2, 0:BT])
    _ld_eng().dma_start(out=ctx0_t[1][:], in_=ap_ctxT[:, 2:4, 0:BT])
    _ld_eng().dma_start(out=u_t[0][:], in_=ap_u[:, 0:4])
    load_data(0, 0)
    load_data(0, 1)
    _ld_eng().dma_start(out=s_sb[:], in_=d_s.ap().rearrange("(rc p) -> p rc", p=P))
    _ld_eng().dma_start(out=u_t[1][:], in_=ap_u[:, 4:8])
    load_data(0, 2)
    _ld_eng().dma_start(out=ctx1[:], in_=ap_ctxT[:, :, BT:])
    _ld_eng().dma_start(out=u_t[2][:], in_=ap_u[:, 8:12])
    load_data(0, 3)
    _ld_eng().dma_start(out=u_t[3][:], in_=ap_u[:, 12:16])
    load_data(0, 4)
    _ld_eng().dma_start(
        out=bias_sb[:], in_=d_bias.ap().rearrange("(mc p) -> p mc", p=P)
    )
    _ld_eng().dma_start(out=vT_sb[:, 0], in_=ap_vT[:, 0])
    load_data(1, 0)
    _ld_eng().dma_start(out=vT_sb[:, 1], in_=ap_vT[:, 1])
    load_data(1, 1)
    load_data(1, 2)
    load_data(1, 3)

    # Per-partition bias operands for the fused evacuation:
    #   scalar engine: relu(psum + bias2)      -> activation(bias=bias2)
    #   vector engine: max(psum, -bias2)+bias2 -> tensor_scalar(max, add)
    # (computed later, after the smod evacs, so the late bias load does not
    # block the scalar engine's FIFO)
    bias2 = singles.tile([P, MC], F32, name="bias2")
    nbias2 = singles.tile([P, MC], F32, name="nbias2")

    # ---- HAM warm-up ---------------------------------------------------
    wu_ps = o_psum.tile([P, BT], F32, tag="po", name="wu_ps")
    for _ in range(N_WARMUP_MM):
        nc.tensor.matmul(wu_ps[:], lhsT=wu_a[:], rhs=wu_b[:], start=True, stop=True)

    def emit_keepers(n):
        for _ in range(n):
            nc.tensor.matmul(
                wu_ps[:, 0:P], lhsT=wu_a[:], rhs=wu_b[:, 0:P], start=True, stop=True
            )

    # ---- compute stages ------------------------------------------------
    pd_t = {}
    smod = singles.tile([P, RC, NB], F32, name="smod")
    lowT = {
        (bt, rc): singles.tile([P, BT], BF16, name=f"lowT{bt}r{rc}")
        for bt in range(NBT)
        for rc in range(RC)
    }

    def emit_rank_mms(bt, g_lo, g_hi, keepers=0):
        """mm1 k-chunks [4*g_lo, 4*g_hi) for both rank chunks."""
        if g_lo == 0:
            pd_t[bt] = [
                du_psum.tile([P, BT], F32, tag="pd", name="pd") for _ in range(RC)
            ]
        for kc in range(4 * g_lo, 4 * g_hi):
            for rc in range(RC):
                nc.tensor.matmul(
                    pd_t[bt][rc][:],
                    lhsT=u_t[kc // 4][:, kc % 4, rc * P : (rc + 1) * P],
                    rhs=data_chunk(bt, kc),
                    start=(kc == 0),
                    stop=(kc == KC - 1),
                )
            if keepers and kc % 2 == 1:
                emit_keepers(keepers)

    def emit_smod(bt):
        """ctx @ w matmuls + s-add; independent of the data stream.

        cc-outer emission for bt0 so the matmuls consume the two ctx halves
        incrementally as they arrive.
        """
        ps_t = [s_psum.tile([P, BT], F32, tag="ps", name="ps") for _ in range(RC)]
        for cc in range(CC):
            for rc in range(RC):
                nc.tensor.matmul(
                    ps_t[rc][:],
                    lhsT=w_sb[:, cc, rc * P : (rc + 1) * P],
                    rhs=ctx_chunk(bt, cc),
                    start=(cc == 0),
                    stop=(cc == CC - 1),
                )
        for rc in range(RC):
            nc.scalar.add(
                smod[:, rc, bt * BT : (bt + 1) * BT],
                ps_t[rc][:],
                add=s_sb[:, rc : rc + 1],
            )

    def emit_mul(bt):
        """lowT = pd * smod on the vector engine (bf16 out)."""
        for rc in range(RC):
            nc.vector.tensor_mul(
                out=lowT[(bt, rc)][:],
                in0=pd_t[bt][rc][:],
                in1=smod[:, rc, bt * BT : (bt + 1) * BT],
            )

    def emit_out_group(bt, g, fine_stores=False):
        """outT[m, b] = relu(vT.T @ lowT + 2*bias) for 4 m-chunks."""
        osb = singles.tile([P, 4, BT], BF16, name=f"o{bt}g{g}")
        for j in range(4):
            mc = 4 * g + j
            # The output stage can rotate over PSUM banks that have retired
            # by the time it runs: bt0 borrows the smod banks (6 deep), bt1
            # additionally borrows the pd banks (8 deep).
            if bt == 1:
                pool = (o_psum, s_psum, o_psum, du_psum)[j]
                tag = ("po", "ps", "po", "pd")[j]
            else:
                pool = (o_psum, s_psum, o_psum, o_psum)[j]
                tag = ("po", "ps", "po", "po")[j]
            po = pool.tile([P, BT], F32, tag=tag, name="po")
            for rc in range(RC):
                nc.tensor.matmul(
                    po[:],
                    lhsT=vT_sb[:, rc, mc * P : (mc + 1) * P],
                    rhs=lowT[(bt, rc)][:],
                    start=(rc == 0),
                    stop=(rc == RC - 1),
                )
            last_mc = fine_stores and bt == 1 and g == 3 and j == 3
            if last_mc:
                # kernel tail: split the final m-chunk's evacuation into two
                # 256-wide halves on the scalar+vector engines in parallel
                # (both idle by now), each stored on its own HWDGE ring, so
                # the post-last-matmul drain chain is halved.
                H = BT // 2
                nc.scalar.activation(
                    osb[:, j, 0:H],
                    po[:, 0:H],
                    mybir.ActivationFunctionType.Relu,
                    bias=bias2[:, mc : mc + 1],
                )
                nc.vector.tensor_scalar(
                    out=osb[:, j, H:],
                    in0=po[:, H:],
                    scalar1=nbias2[:, mc : mc + 1],
                    scalar2=bias2[:, mc : mc + 1],
                    op0=mybir.AluOpType.max,
                    op1=mybir.AluOpType.add,
                )
                nc.scalar.dma_start(
                    out=ap_outT[:, mc, bt * BT : bt * BT + H], in_=osb[:, j, 0:H]
                )
                nc.sync.dma_start(
                    out=ap_outT[:, mc, bt * BT + H : (bt + 1) * BT],
                    in_=osb[:, j, H:],
                )
                continue
            if mc % 2 == 0:
                nc.scalar.activation(
                    osb[:, j, :],
                    po[:],
                    mybir.ActivationFunctionType.Relu,
                    bias=bias2[:, mc : mc + 1],
                )
            else:
                nc.vector.tensor_scalar(
                    out=osb[:, j, :],
                    in0=po[:],
                    scalar1=nbias2[:, mc : mc + 1],
                    scalar2=bias2[:, mc : mc + 1],
                    op0=mybir.AluOpType.max,
                    op1=mybir.AluOpType.add,
                )
            if fine_stores:
                eng = nc.scalar if mc % 2 == 0 else nc.sync
                eng.dma_start(
                    out=ap_outT[:, mc, bt * BT : (bt + 1) * BT], in_=osb[:, j, :]
                )
        if not fine_stores:
            nc.gpsimd.dma_start(
                out=ap_outT[:, 4 * g : 4 * g + 4, bt * BT : (bt + 1) * BT],
                in_=osb[:],
            )

    # Software pipeline, PE emission ordered to match DMA arrival order;
    # batch-tile 1's rank stage interleaves with batch-tile 0's output
    # stage so the PE never waits on the load stream's tail.
    emit_smod(0)
    emit_rank_mms(0, 0, 2, keepers=1)
    emit_smod(1)
    emit_rank_mms(0, 2, 4, keepers=1)
    emit_mul(0)
    nc.scalar.mul(bias2[:], bias_sb[:], 2.0)
    nc.scalar.mul(nbias2[:], bias_sb[:], -2.0)
    emit_out_group(0, 0)
    emit_rank_mms(1, 0, 1)
    emit_out_group(0, 1)
    emit_rank_mms(1, 1, 2)
    emit_out_group(0, 2)
    emit_rank_mms(1, 2, 3)
    emit_out_group(0, 3)
    emit_rank_mms(1, 3, 4)
    emit_mul(1)
    emit_out_group(1, 0)
    emit_out_group(1, 1)
    emit_out_group(1, 2, fine_stores=True)
    emit_out_group(1, 3, fine_stores=True)


_CACHE = {}


def build():
    if "nc" in _CACHE:
        return _CACHE["nc"]
    nc = bacc.Bacc("TRN2", target_bir_lowering=False, debug=False)
    with tile.TileContext(nc) as tc, ExitStack() as ctx:
        _emit(nc, tc, ctx)
    nc.compile()
    _CACHE["nc"] = nc
    return nc


def make_in_maps(data, context, u, s, v, w, bias):
    u_b = np.ascontiguousarray(np.asarray(u, dtype=np.float32)).astype(NP_BF16)
    s = np.ascontiguousarray(np.asarray(s, dtype=np.float32))
    vT_b = np.ascontiguousarray(np.asarray(v, dtype=np.float32).T).astype(NP_BF16)
    w_b = np.ascontiguousarray(np.asarray(w, dtype=np.float32)).astype(NP_BF16)
    bias = np.ascontiguousarray(np.asarray(bias, dtype=np.float32))
    data = np.asarray(data, dtype=np.float32)
    context = np.asarray(context, dtype=np.float32)
    in_maps = []
    for c in range(NCORES):
        sl = slice(c * NB, (c + 1) * NB)
        in_maps.append(
            {
                "dataT": np.ascontiguousarray(data[sl].T).astype(NP_BF16),
                "ctxT": np.ascontiguousarray(context[sl].T).astype(NP_BF16),
                "u": u_b,
                "s": s,
                "vT": vT_b,
                "w": w_b,
                "bias": bias,
            }
        )
    return in_maps


def kernel(data, context, u, s, v, w, bias):
    nc = build()
    in_maps = make_in_maps(data, context, u, s, v, w, bias)
    res = run_bass_kernel_spmd(nc, in_maps, core_ids=list(range(NCORES)))
    return np.concatenate(
        [np.asarray(r["outT"]).astype(np.float32).T for r in res.results], axis=0
    )


# revision 61
# speedup vs baseline: 1.0113x; 1.0113x over previous
"""Trainium2 Bass kernel for nn_CADense (context-adaptive low-rank dense layer).

Computes, for the full batch:
    s_mod = s + context @ w          # [B, R]
    low   = (data @ u) * s_mod       # [B, R]
    out   = relu(low @ v.T + 2*bias) # [B, UNITS]

Sharding: data-parallel over batch across 8 NeuronCores; u/s/v/w/bias
replicated. Each core runs the same Bass program on its 1024-row shard.

The kernel is DMA-bound at fp32 (22.5 MiB/core vs the ~358 GB/s per-core
HBM limit), so the whole datapath runs in bf16: inputs are cast host-side,
matmuls are bf16 with fp32 PSUM accumulation, and the output is stored as
bf16 and upcast host-side. That halves HBM traffic to ~11.3 MiB/core,
which sits right at the PE streaming floor (~31 us) — the ridge point.

All compute stays in the transposed domain:
    pd[r, b]   = (u.T @ data.T)[r, b]                 (PE, fp32 PSUM)
    smod[r, b] = s[r] + (w.T @ ctx.T)[r, b]           (PE + scalar add)
    lowT[r, b] = pd * smod                            (DVE, writes bf16)
    outT[m, b] = relu((vT.T @ lowT)[m, b] + 2*bias[m])
The output stays transposed so the 2*bias term is a per-partition scalar:
it rides for free inside the PSUM-evacuation op (scalar.activation bias,
or DVE tensor_scalar max/add), eliminating the 32 K=1 bias matmuls the
natural-layout version needs (~7 us of pure PE streaming waste).

Schedule notes:
- Input DMAs ride the sync HWDGE ring in first-consumption order (the SP
  sequencer has no other duties, so ring-full stalls are free there);
  s/w/ctx go first so the smod stage doubles as real PE warm-up work,
  and the tiny s/bias transfers sit mid-stream where their ~2us
  completion latency hides under the pipelined bulk loads.
- Batch-tile 1's rank stage interleaves with batch-tile 0's output stage
  in PE emission order so the PE never waits on the load stream's tail.
- Output stores ride the gpsimd SWDGE ring so they round-robin against
  pending loads at the SDMA packet level; the last two groups store per-
  m-chunk on the two HWDGE rings so the final SWDGE drain isn't gated
  by a late store receipt (it was a 5.6us drain otherwise).
- PSUM evacuation of the 32 output groups alternates between the scalar
  and vector engines so neither gates PSUM bank recycling; batch-tile
  1's output groups rotate over all 8 PSUM banks (pd/smod banks have
  retired by then).
- A few bf16 garbage matmuls pre-warm the HAM clock gate while the first
  loads stream in; keepers sprinkled into the first rank stage cover the
  initial DMA-paced bubbles.
"""

import os
import sys
from contextlib import ExitStack

import numpy as np
import ml_dtypes


def _ensure_concourse():
    try:
        import concourse  # noqa: F401
    except ImportError:
        for p in ("/opt/trn_rl_repo", "/root/.axon_site/_ro/trn_rl_repo"):
            if os.path.isdir(p) and p not in sys.path:
                sys.path.insert(0, p)


_ensure_concourse()

import concourse.tile as tile  # noqa: E402
from concourse import bacc, mybir  # noqa: E402
from concourse.bass_utils import run_bass_kernel_spmd  # noqa: E402

NCORES = 8
B, N_IN, UNITS, RANK, CCTX = 8192, 2048, 2048, 256, 512
NB = B // NCORES  # batch rows per core
P = 128
BT = 512  # batch tile (free dim of matmuls)
NBT = NB // BT  # 2 batch tiles per core
KC = N_IN // P  # 16 contraction chunks for data @ u
CC = CCTX // P  # 4 contraction chunks for context @ w
RC = RANK // P  # 2 rank chunks
MC = UNITS // P  # 16 output unit chunks (partition dim of outT)
N_WARMUP_MM = 11

F32 = mybir.dt.float32
BF16 = mybir.dt.bfloat16
NP_BF16 = np.dtype(ml_dtypes.bfloat16)


def _emit(nc, tc, ctx):
    d_dataT = nc.dram_tensor("dataT", [N_IN, NB], BF16, kind="ExternalInput")
    d_ctxT = nc.dram_tensor("ctxT", [CCTX, NB], BF16, kind="ExternalInput")
    d_u = nc.dram_tensor("u", [N_IN, RANK], BF16, kind="ExternalInput")
    d_s = nc.dram_tensor("s", [RANK], F32, kind="ExternalInput")
    d_vT = nc.dram_tensor("vT", [RANK, UNITS], BF16, kind="ExternalInput")
    d_w = nc.dram_tensor("w", [CCTX, RANK], BF16, kind="ExternalInput")
    d_bias = nc.dram_tensor("bias", [UNITS], F32, kind="ExternalInput")
    d_outT = nc.dram_tensor("outT", [UNITS, NB], BF16, kind="ExternalOutput")

    ap_dataT = d_dataT.ap().rearrange("(c p) b -> p c b", p=P)
    ap_ctxT = d_ctxT.ap().rearrange("(cc p) b -> p cc b", p=P)
    ap_u = d_u.ap().rearrange("(q p) r -> p q r", p=P)
    ap_w = d_w.ap().rearrange("(cc p) r -> p cc r", p=P)
    ap_vT = d_vT.ap().rearrange("(rc p) m -> p rc m", p=P)
    ap_outT = d_outT.ap().rearrange("(mc p) b -> p mc b", p=P)

    singles = ctx.enter_context(tc.tile_pool(name="singles", bufs=1))
    du_psum = ctx.enter_context(tc.tile_pool(name="du_psum", bufs=2, space="PSUM"))
    s_psum = ctx.enter_context(tc.tile_pool(name="s_psum", bufs=2, space="PSUM"))
    o_psum = ctx.enter_context(tc.tile_pool(name="o_psum", bufs=4, space="PSUM"))

    # HAM warm-up fodder: garbage bf16 matmuls while the first loads stream.
    wu_a = singles.tile([P, P], BF16)
    nc.vector.memset(wu_a[:], 1.0)
    wu_b = singles.tile([P, BT], BF16)
    nc.vector.memset(wu_b[:], 1.0)

    # ---- input DMA queue (sync HWDGE ring), in first-use order ----------
    # Tiny transfers (s/bias/w) ride mid-stream where their ~2us completion
    # latency hides under the pipelined bulk loads; putting them first costs
    # ~5us of dead time at the head.
    u_t = {}  # u_t[g][:, j, :] covers kc = 4*g + j
    for g in range(4):
        u_t[g] = singles.tile([P, 4, RANK], BF16, name=f"u{g}")
    # dataT chunks: the first batch-tile-0 chunk is split in 2-kc halves so
    # the DMA-paced rank stage starts on a finer-grained completion; the
    # rest are 4-kc pieces (finer everywhere costs descriptor overhead).
    D0_CHUNKS = [(0, 2), (2, 2), (4, 4), (8, 4), (12, 4)]  # (kc_lo, width)
    d_t = {}
    for g, (kc_lo, w_kc) in enumerate(D0_CHUNKS):
        d_t[(0, g)] = singles.tile([P, w_kc, BT], BF16, name=f"d0g{g}")
    for g in range(4):
        d_t[(1, g)] = singles.tile([P, 4, BT], BF16, name=f"d1g{g}")

    def data_chunk(bt, kc):
        if bt == 0:
            for g, (kc_lo, w_kc) in enumerate(D0_CHUNKS):
                if kc_lo <= kc < kc_lo + w_kc:
                    return d_t[(0, g)][:, kc - kc_lo, :]
        return d_t[(1, kc // 4)][:, kc % 4, :]

    # All loads on the sync (SP) HWDGE ring: the SP sequencer has no other
    # duties, so it can absorb ring-full stalls that would block the ACT
    # engine's compute FIFO.
    def _ld_eng():
        return nc.sync

    def load_data(bt, g):
        if bt == 0:
            kc_lo, w_kc = D0_CHUNKS[g]
        else:
            kc_lo, w_kc = 4 * g, 4
        _ld_eng().dma_start(
            out=d_t[(bt, g)][:],
            in_=ap_dataT[:, kc_lo : kc_lo + w_kc, bt * BT : (bt + 1) * BT],
        )

    s_sb = singles.tile([P, RC], F32, name="s_sb")
    bias_sb = singles.tile([P, MC], F32, name="bias_sb")
    w_sb = singles.tile([P, CC, RANK], BF16, name="w_sb")
    # ctx(bt0) in two 2-cc halves so the smod stage can start on the first
    # half's arrival; ctx(bt1) rides mid-stream as one piece.
    ctx0_t = {
        h: singles.tile([P, 2, BT], BF16, name=f"ctx0h{h}") for h in range(2)
    }
    ctx1 = singles.tile([P, CC, BT], BF16, name="ctx1")
    vT_sb = singles.tile([P, RC, UNITS], BF16, name="vT_sb")

    def ctx_chunk(bt, cc):
        if bt == 0:
            return ctx0_t[cc // 2][:, cc % 2, :]
        return ctx1[:, cc, :]

    # w/ctx(bt0) first: the smod stage becomes real warm-up work for the
    # PE while the data stream ramps up; the tiny s load rides mid-stream
    # (needed only at the smod PSUM evacuation).
    _ld_eng().dma_start(out=w_sb[:], in_=ap_w)
    _ld_eng().dma_start(out=ctx0_t[0][:], in_=ap_ctxT[:, 0:2, 0:BT])
    _ld_eng().dma_start(out=ctx0_t[1][:], in_=ap_ctxT[:, 2:4, 0:BT])
    _ld_eng().dma_start(out=u_t[0][:], in_=ap_u[:, 0:4])
    load_data(0, 0)
    load_data(0, 1)
    _ld_eng().dma_start(out=s_sb[:], in_=d_s.ap().rearrange("(rc p) -> p rc", p=P))
    _ld_eng().dma_start(out=u_t[1][:], in_=ap_u[:, 4:8])
    load_data(0, 2)
    _ld_eng().dma_start(out=ctx1[:], in_=ap_ctxT[:, :, BT:])
    _ld_eng().dma_start(out=u_t[2][:], in_=ap_u[:, 8:12])
    load_data(0, 3)
    _ld_eng().dma_start(out=u_t[3][:], in_=ap_u[:, 12:16])
    load_data(0, 4)
    _ld_eng().dma_start(
        out=bias_sb[:], in_=d_bias.ap().rearrange("(mc p) -> p mc", p=P)
    )
    _ld_eng().dma_start(out=vT_sb[:, 0], in_=ap_vT[:, 0])
    load_data(1, 0)
    _ld_eng().dma_start(out=vT_sb[:, 1], in_=ap_vT[:, 1])
    load_data(1, 1)
    load_data(1, 2)
    load_data(1, 3)

    # Per-partition bias operands for the fused evacuation:
    #   scalar engine: relu(psum + bias2)      -> activation(bias=bias2)
    #   vector engine: max(psum, -bias2)+bias2 -> tensor_scalar(max, add)
    # (computed later, after the smod evacs, so the late bias load does not
    # block the scalar engine's FIFO)
    bias2 = singles.tile([P, MC], F32, name="bias2")
    nbias2 = singles.tile([P, MC], F32, name="nbias2")

    # ---- HAM warm-up ---------------------------------------------------
    wu_ps = o_psum.tile([P, BT], F32, tag="po", name="wu_ps")
    for _ in range(N_WARMUP_MM):
        nc.tensor.matmul(wu_ps[:], lhsT=wu_a[:], rhs=wu_b[:], start=True, stop=True)

    def emit_keepers(n):
        for _ in range(n):
            nc.tensor.matmul(
                wu_ps[:, 0:P], lhsT=wu_a[:], rhs=wu_b[:, 0:P], start=True, stop=True
            )

    # ---- compute stages ------------------------------------------------
    pd_t = {}
    smod = singles.tile([P, RC, NB], F32, name="smod")
    lowT = {
        (bt, rc): singles.tile([P, BT], BF16, name=f"lowT{bt}r{rc}")
        for bt in range(NBT)
        for rc in range(RC)
    }

    def emit_rank_mms(bt, g_lo, g_hi, keepers=0):
        """mm1 k-chunks [4*g_lo, 4*g_hi) for both rank chunks."""
        if g_lo == 0:
            pd_t[bt] = [
                du_psum.tile([P, BT], F32, tag="pd", name="pd") for _ in range(RC)
            ]
        for kc in range(4 * g_lo, 4 * g_hi):
            for rc in range(RC):
                nc.tensor.matmul(
                    pd_t[bt][rc][:],
                    lhsT=u_t[kc // 4][:, kc % 4, rc * P : (rc + 1) * P],
                    rhs=data_chunk(bt, kc),
                    start=(kc == 0),
                    stop=(kc == KC - 1),
                )
            if keepers and kc % 2 == 1:
                emit_keepers(keepers)

    def emit_smod(bt):
        """ctx @ w matmuls + s-add; independent of the data stream.

        cc-outer emission for bt0 so the matmuls consume the two ctx halves
        incrementally as they arrive.
        """
        ps_t = [s_psum.tile([P, BT], F32, tag="ps", name="ps") for _ in range(RC)]
        for cc in range(CC):
            for rc in range(RC):
                nc.tensor.matmul(
                    ps_t[rc][:],
                    lhsT=w_sb[:, cc, rc * P : (rc + 1) * P],
                    rhs=ctx_chunk(bt, cc),
                    start=(cc == 0),
                    stop=(cc == CC - 1),
                )
        for rc in range(RC):
            nc.scalar.add(
                smod[:, rc, bt * BT : (bt + 1) * BT],
                ps_t[rc][:],
                add=s_sb[:, rc : rc + 1],
            )

    def emit_mul(bt):
        """lowT = pd * smod on the vector engine (bf16 out)."""
        for rc in range(RC):
            nc.vector.tensor_mul(
                out=lowT[(bt, rc)][:],
                in0=pd_t[bt][rc][:],
                in1=smod[:, rc, bt * BT : (bt + 1) * BT],
            )

    def emit_out_group(bt, g, fine_stores=False):
        """outT[m, b] = relu(vT.T @ lowT + 2*bias) for 4 m-chunks."""
        osb = singles.tile([P, 4, BT], BF16, name=f"o{bt}g{g}")
        for j in range(4):
            mc = 4 * g + j
            # The output stage can rotate over PSUM banks that have retired
            # by the time it runs: bt0 borrows the smod banks (6 deep), bt1
            # additionally borrows the pd banks (8 deep).
            if bt == 1:
                pool = (o_psum, s_psum, o_psum, du_psum)[j]
                tag = ("po", "ps", "po", "pd")[j]
            else:
                pool = (o_psum, s_psum, o_psum, o_psum)[j]
                tag = ("po", "ps", "po", "po")[j]
            po = pool.tile([P, BT], F32, tag=tag, name="po")
            for rc in range(RC):
                nc.tensor.matmul(
                    po[:],
                    lhsT=vT_sb[:, rc, mc * P : (mc + 1) * P],
                    rhs=lowT[(bt, rc)][:],
                    start=(rc == 0),
                    stop=(rc == RC - 1),
                )
            if mc % 2 == 0:
                nc.scalar.activation(
                    osb[:, j, :],
                    po[:],
                    mybir.ActivationFunctionType.Relu,
                    bias=bias2[:, mc : mc + 1],
                )
            else:
                nc.vector.tensor_scalar(
                    out=osb[:, j, :],
                    in0=po[:],
                    scalar1=nbias2[:, mc : mc + 1],
                    scalar2=bias2[:, mc : mc + 1],
                    op0=mybir.AluOpType.max,
                    op1=mybir.AluOpType.add,
                )
            if fine_stores:
                eng = nc.scalar if mc % 2 == 0 else nc.sync
                eng.dma_start(
                    out=ap_outT[:, mc, bt * BT : (bt + 1) * BT], in_=osb[:, j, :]
                )
        if not fine_stores:
            nc.gpsimd.dma_start(
                out=ap_outT[:, 4 * g : 4 * g + 4, bt * BT : (bt + 1) * BT],
                in_=osb[:],
            )

    # Software pipeline, PE emission ordered to match DMA arrival order;
    # batch-tile 1's rank stage interleaves with batch-tile 0's output
    # stage so the PE never waits on the load stream's tail.
    emit_smod(0)
    emit_rank_mms(0, 0, 2, keepers=1)
    emit_smod(1)
    emit_rank_mms(0, 2, 4, keepers=1)
    emit_mul(0)
    nc.scalar.mul(bias2[:], bias_sb[:], 2.0)
    nc.scalar.mul(nbias2[:], bias_sb[:], -2.0)
    emit_out_group(0, 0)
    emit_rank_mms(1, 0, 1)
    emit_out_group(0, 1)
    emit_rank_mms(1, 1, 2)
    emit_out_group(0, 2)
    emit_rank_mms(1, 2, 3)
    emit_out_group(0, 3)
    emit_rank_mms(1, 3, 4)
    emit_mul(1)
    emit_out_group(1, 0)
    emit_out_group(1, 1)
    emit_out_group(1, 2, fine_stores=True)
    emit_out_group(1, 3, fine_stores=True)


_CACHE = {}


def build():
    if "nc" in _CACHE:
        return _CACHE["nc"]
    nc = bacc.Bacc("TRN2", target_bir_lowering=False, debug=False)
    with tile.TileContext(nc) as tc, ExitStack() as ctx:
        _emit(nc, tc, ctx)
    nc.compile()
    _CACHE["nc"] = nc
    return nc


def make_in_maps(data, context, u, s, v, w, bias):
    u_b = np.ascontiguousarray(np.asarray(u, dtype=np.float32)).astype(NP_BF16)
    s = np.ascontiguousarray(np.asarray(s, dtype=np.float32))
    vT_b = np.ascontiguousarray(np.asarray(v, dtype=np.float32).T).astype(NP_BF16)
    w_b = np.ascontiguousarray(np.asarray(w, dtype=np.float32)).astype(NP_BF16)
    bias = np.ascontiguousarray(np.asarray(bias, dtype=np.float32))
    data = np.asarray(data, dtype=np.float32)
    context = np.asarray(context, dtype=np.float32)
    in_maps = []
    for c in range(NCORES):
        sl = slice(c * NB, (c + 1) * NB)
        in_maps.append(
            {
                "dataT": np.ascontiguousarray(data[sl].T).astype(NP_BF16),
                "ctxT": np.ascontiguousarray(context[sl].T).astype(NP_BF16),
                "u": u_b,
                "s": s,
                "vT": vT_b,
                "w": w_b,
                "bias": bias,
            }
        )
    return in_maps


def kernel(data, context, u, s, v, w, bias):
    nc = build()
    in_maps = make_in_maps(data, context, u, s, v, w, bias)
    res = run_bass_kernel_spmd(nc, in_maps, core_ids=list(range(NCORES)))
    return np.concatenate(
        [np.asarray(r["outT"]).astype(np.float32).T for r in res.results], axis=0
    )
